# revision 8
# baseline (speedup 1.0000x reference)
"""GCN (2-layer, hidden=64, rank-1 weights) on 8 Trainium2 NeuronCores.

Math: both GCNConv layers have rank-1 weight matrices (1->64, 64->1), so each
layer collapses to a scalar SpMV with the symmetric-normalized adjacency
A_hat = D^-1/2 (A+I) D^-1/2:

    s   = A_hat @ x                    (scalar per node)
    z   = f(s)   where f(t) = sum_k W2[k] * relu(W1[k]*t + b1[k])
    out = A_hat @ z + b2

Sharding: nodes are range-sharded by destination across the 8 cores; all
in-edges of a node live on its owner core.  Within a core, nodes are sorted
by in-degree (descending) so that "round r" (the r-th in-edge of every node
that has one) is a dense prefix of node slots -- the edge-routed per-slot
value arrays are therefore nearly pad-free (ELL with degree-sorted rounds).

Execution is two SPMD launches (one per GCN layer).  The host routes
per-edge source data to the owning destination core between layers (np.take
-- pure gather, the "halo exchange" of the sharding strategy).  Routed
per-edge tables are fp16 so the DVE's 2-byte fast path applies and HBM
traffic is halved.  Normalization coefficients dinv = 1/sqrt(deg+1) are a
function of the graph structure only (host already derives degrees with
np.bincount to build the routing tables), so the host routes dinv[src]
per-edge directly; the device performs all feature arithmetic: the per-edge
message products dinv[src]*x[src], the segment summation (fold-tree reduce
over the ELL tile), the MLP nonlinearity (weight-folded to a 2-segment
piecewise-linear map when b1 == 0), the per-node normalization, the layer-2
message values w = dinv*z, and the bias.

Only the vector (DVE) and sync engines are used: relu is a tensor_scalar_max
on DVE, so no ACT table load, and intra-engine ordering relies on the DVE's
in-order execution (no per-op semaphore churn).  Each layer's big routed
table is DMA'd in two column-halves so the first half's compute overlaps the
second half's transfer.
"""

import os
import numpy as np

from concourse import bass, mybir
from concourse.bass_utils import run_bass_kernel_spmd

dt = mybir.dt
F16 = np.float16

NCORES = 8
N = 100000
P = 128            # SBUF partitions
CPN = 98           # node columns per partition
NPC = P * CPN      # 12544 nodes per core
SENT = NCORES * NPC  # sentinel table slot (value 0)

LAST_RESULTS = None  # list of BassKernelResults from the most recent run


def _preprocess(x, edge_index):
    """Host routing/layout: shard by destination, degree-sort nodes, build
    per-slot source-index arrays (ELL with degree-sorted rounds)."""
    x = np.asarray(x, dtype=np.float32).reshape(-1)
    ei = np.asarray(edge_index)
    src_g = ei[0].astype(np.int64)
    dst_g = ei[1].astype(np.int64)

    cnt_g = np.bincount(dst_g, minlength=N).astype(np.int64)  # in-degree

    order_c, rank_c, deg_sorted_c = [], [], []
    pp = np.empty(N, dtype=np.int64)  # global node -> permuted table position
    for c in range(NCORES):
        lo, hi = c * NPC, min((c + 1) * NPC, N)
        nreal = hi - lo
        deg_local = np.zeros(NPC, dtype=np.int64)
        deg_local[:nreal] = cnt_g[lo:hi]
        order = np.argsort(-deg_local, kind="stable")
        rank = np.empty(NPC, dtype=np.int64)
        rank[order] = np.arange(NPC)
        order_c.append(order)
        rank_c.append(rank)
        deg_sorted_c.append(deg_local[order])
        pp[lo:hi] = c * NPC + rank[:nreal]

    K = int(max(int(d[0]) for d in deg_sorted_c))  # global max in-degree

    owner = dst_g // NPC
    idx_c, xs_c, dinv_c = [], [], []
    for c in range(NCORES):
        lo = c * NPC
        m = owner == c
        s_e = pp[src_g[m]]
        d_e = dst_g[m] - lo
        rj = rank_c[c][d_e]
        o = np.argsort(rj, kind="stable")
        rj_s = rj[o]
        s_s = s_e[o]
        occ = np.arange(len(rj_s)) - np.searchsorted(rj_s, rj_s)
        idx_mat = np.full((NPC, K), SENT, dtype=np.int64)
        idx_mat[rj_s, occ] = s_s
        # SBUF layout [p, r*98 + cc] for node j = p*98 + cc
        idx_c.append(np.ascontiguousarray(
            idx_mat.reshape(P, CPN, K).transpose(0, 2, 1).reshape(P, K * CPN)))

        nreal = min(NPC, N - lo)
        xv = np.zeros(NPC, dtype=np.float32)
        xv[:nreal] = x[lo:lo + nreal]
        xs_c.append(np.ascontiguousarray(
            xv[order_c[c]].astype(np.float32).reshape(P, CPN)))
        dinv_c.append(np.ascontiguousarray(
            (1.0 / np.sqrt(deg_sorted_c[c] + 1.0)).astype(np.float32)
            .reshape(P, CPN)))
    return idx_c, xs_c, dinv_c, rank_c, K


def _emit_tree(vector, Y, lo, nb):
    """In-place fold-tree over `nb` CPN-wide column blocks of Y starting at
    block `lo`; the sum lands in block `lo`.  All slices are contiguous."""
    w = nb
    while w > 1:
        h = (w + 1) // 2
        vector.tensor_tensor(
            out=Y[:, lo * CPN:(lo + w - h) * CPN],
            in0=Y[:, lo * CPN:(lo + w - h) * CPN],
            in1=Y[:, (lo + h) * CPN:(lo + w) * CPN],
            op=mybir.AluOpType.add)
        w = h


def _build_layer1(K, A, B, terms):
    """Layer 1: routed per-edge tables [x[src] | dinv[src]] (fp16, in two
    column-halves), per-node [x_own | dinv_own] (f32).
    Output: w_own = dinv * f(s)  [the routed message value for layer 2]."""
    nc = bass.Bass(num_devices=NCORES)
    hA = (K + 1) // 2
    cA, cB = hA * CPN, (K - hA) * CPN

    ea_in = nc.declare_dram_parameter("en_a", [P, 2 * cA], dt.float16, isOutput=False)
    eb_in = nc.declare_dram_parameter("en_b", [P, 2 * cB], dt.float16, isOutput=False)
    po_in = nc.declare_dram_parameter("po", [P, 2 * CPN], dt.float32, isOutput=False)
    out_ext = nc.declare_dram_parameter("out", [P, CPN], dt.float32, isOutput=True)

    with (
        nc.sbuf_tensor("EA", [P, 2 * cA], dt.float16) as EA,
        nc.sbuf_tensor("EB", [P, 2 * cB], dt.float16) as EB,
        nc.sbuf_tensor("Y", [P, K * CPN], dt.float16) as Y,
        nc.sbuf_tensor("PO", [P, 2 * CPN], dt.float32) as PO,
        nc.sbuf_tensor("tb", [P, CPN], dt.float32) as tb,
        nc.sbuf_tensor("ts", [P, CPN], dt.float32) as ts,
        nc.sbuf_tensor("tr", [P, CPN], dt.float32) as tr,
        nc.sbuf_tensor("to", [P, CPN], dt.float32) as to,
        nc.semaphore("sa") as sa,
        nc.semaphore("sb") as sb,
        nc.semaphore("sp") as sp,
        nc.semaphore("sv") as sv,
        nc.Block(no_gpsimd_drain=True) as block,
    ):
        @block.scalar
        def _(scalar):
            scalar.dma_start(out=EB[:, :], in_=eb_in[:, :]).then_inc(sb, 16)

        @block.gpsimd
        def _(gpsimd):
            gpsimd.dma_start(out=PO[:, :], in_=po_in[:, :]).then_inc(sp, 16)

        @block.vector
        def _(vector):
            xo = PO[:, 0:CPN]
            do = PO[:, CPN:2 * CPN]
            # per-edge messages y = dinv[src] * x[src], all fp16
            vector.wait_ge(sa, 16)
            vector.tensor_tensor(
                out=Y[:, 0:cA], in0=EA[:, 0:cA], in1=EA[:, cA:2 * cA],
                op=mybir.AluOpType.mult)
            _emit_tree(vector, Y, 0, hA)
            vector.wait_ge(sb, 16)
            vector.tensor_tensor(
                out=Y[:, cA:cA + cB], in0=EB[:, 0:cB], in1=EB[:, cB:2 * cB],
                op=mybir.AluOpType.mult)
            _emit_tree(vector, Y, hA, K - hA)
            vector.tensor_tensor(
                out=Y[:, 0:CPN], in0=Y[:, 0:CPN], in1=Y[:, hA * CPN:(hA + 1) * CPN],
                op=mybir.AluOpType.add)
            # s = dinv * (fold + dinv * x_own)
            vector.wait_ge(sp, 16)
            vector.tensor_tensor(out=tb[:, :], in0=do, in1=xo,
                                 op=mybir.AluOpType.mult)
            vector.tensor_tensor(out=tb[:, :], in0=tb[:, :], in1=Y[:, 0:CPN],
                                 op=mybir.AluOpType.add)
            vector.tensor_tensor(out=ts[:, :], in0=do, in1=tb[:, :],
                                 op=mybir.AluOpType.mult)
            if terms is None:
                # z = (A-B)*relu(s) + B*s
                vector.tensor_scalar_max(tr[:, :], ts[:, :], 0.0)
                vector.tensor_scalar_mul(to[:, :], tr[:, :], float(A - B))
                vector.scalar_tensor_tensor(
                    out=to[:, :], in0=ts[:, :], scalar=float(B), in1=to[:, :],
                    op0=mybir.AluOpType.mult, op1=mybir.AluOpType.add)
            else:
                vector.memset(to[:, :], 0.0)
                for (w1k, b1k, w2k) in terms:
                    vector.tensor_scalar(
                        tr[:, :], ts[:, :], float(w1k), float(b1k),
                        mybir.AluOpType.mult, mybir.AluOpType.add)
                    vector.tensor_scalar_max(tr[:, :], tr[:, :], 0.0)
                    vector.scalar_tensor_tensor(
                        out=to[:, :], in0=tr[:, :], scalar=float(w2k),
                        in1=to[:, :],
                        op0=mybir.AluOpType.mult, op1=mybir.AluOpType.add)
            # w_own = dinv * z
            vector.tensor_tensor(
                out=to[:, :], in0=do, in1=to[:, :],
                op=mybir.AluOpType.mult).then_inc(sv, 1)

        @block.sync
        def _(sync):
            sync.dma_start(out=EA[:, :], in_=ea_in[:, :]).then_inc(sa, 16)
            sync.wait_ge(sv, 1)
            sync.dma_start(out=out_ext[:, :], in_=to[:, :]).then_inc(sp, 16)

    return nc


def _build_layer2(K, b2val):
    """Layer 2: routed per-edge table w[src] (fp16, two column-halves; w is
    the device-computed dinv*z), per-node [w_own | dinv_own] (f32).
    out = dinv*(sum w_ell + w_own) + b2."""
    nc = bass.Bass(num_devices=NCORES)
    hA = (K + 1) // 2
    cA, cB = hA * CPN, (K - hA) * CPN

    wa_in = nc.declare_dram_parameter("we_a", [P, cA], dt.float16, isOutput=False)
    wb_in = nc.declare_dram_parameter("we_b", [P, cB], dt.float16, isOutput=False)
    po_in = nc.declare_dram_parameter("po", [P, 2 * CPN], dt.float32, isOutput=False)
    out_ext = nc.declare_dram_parameter("out", [P, CPN], dt.float32, isOutput=True)

    with (
        nc.sbuf_tensor("WA", [P, cA], dt.float16) as WA,
        nc.sbuf_tensor("WB", [P, cB], dt.float16) as WB,
        nc.sbuf_tensor("PO", [P, 2 * CPN], dt.float32) as PO,
        nc.sbuf_tensor("tb", [P, CPN], dt.float32) as tb,
        nc.sbuf_tensor("to", [P, CPN], dt.float32) as to,
        nc.semaphore("sa") as sa,
        nc.semaphore("sb") as sb,
        nc.semaphore("sp") as sp,
        nc.semaphore("sv") as sv,
        nc.Block(no_gpsimd_drain=True) as block,
    ):
        @block.scalar
        def _(scalar):
            scalar.dma_start(out=WB[:, :], in_=wb_in[:, :]).then_inc(sb, 16)

        @block.gpsimd
        def _(gpsimd):
            gpsimd.dma_start(out=PO[:, :], in_=po_in[:, :]).then_inc(sp, 16)

        @block.vector
        def _(vector):
            wo = PO[:, 0:CPN]
            do = PO[:, CPN:2 * CPN]
            vector.wait_ge(sa, 16)
            _emit_tree(vector, WA, 0, hA)
            vector.wait_ge(sb, 16)
            _emit_tree(vector, WB, 0, K - hA)
            vector.tensor_tensor(
                out=tb[:, :], in0=WA[:, 0:CPN], in1=WB[:, 0:CPN],
                op=mybir.AluOpType.add)
            vector.wait_ge(sp, 16)
            vector.tensor_tensor(out=tb[:, :], in0=tb[:, :], in1=wo,
                                 op=mybir.AluOpType.add)
            vector.tensor_tensor(out=tb[:, :], in0=do, in1=tb[:, :],
                                 op=mybir.AluOpType.mult)
            vector.tensor_scalar_add(to[:, :], tb[:, :],
                                     float(b2val)).then_inc(sv, 1)

        @block.sync
        def _(sync):
            sync.dma_start(out=WA[:, :], in_=wa_in[:, :]).then_inc(sa, 16)
            sync.wait_ge(sv, 1)
            sync.dma_start(out=out_ext[:, :], in_=to[:, :]).then_inc(sp, 16)

    return nc


def kernel(x, edge_index, W1, b1, W2, b2):
    global LAST_RESULTS
    idx_c, xs_c, dinv_c, rank_c, K = _preprocess(x, edge_index)
    hA = (K + 1) // 2
    cA = hA * CPN

    w1 = np.asarray(W1, dtype=np.float64).reshape(-1)
    w2 = np.asarray(W2, dtype=np.float64).reshape(-1)
    b1v = np.asarray(b1, dtype=np.float64).reshape(-1)
    b2v = float(np.asarray(b2, dtype=np.float64).reshape(-1)[0])
    if np.all(b1v == 0.0):
        A = float(np.sum(w2 * w1 * (w1 > 0)))
        B = float(np.sum(w2 * w1 * (w1 < 0)))
        terms = None
    else:
        A = B = 0.0
        terms = [(float(w1[k]), float(b1v[k]), float(w2[k]))
                 for k in range(len(w1))]

    # routed tables in permuted (per-core degree-sorted) order + sentinel 0
    x_tab = np.zeros(SENT + 1, dtype=np.float32)
    d_tab = np.zeros(SENT + 1, dtype=np.float32)
    for c in range(NCORES):
        x_tab[c * NPC:(c + 1) * NPC] = xs_c[c].reshape(-1)
        d_tab[c * NPC:(c + 1) * NPC] = dinv_c[c].reshape(-1)
    x_tab16 = x_tab.astype(F16)
    d_tab16 = d_tab.astype(F16)

    trace = bool(os.environ.get("BASS_TRACE"))

    # ---- layer 1 ----
    nc1 = _build_layer1(K, A, B, terms)
    maps1 = []
    for c in range(NCORES):
        ia, ib = idx_c[c][:, :cA], idx_c[c][:, cA:]
        maps1.append({
            "en_a": np.ascontiguousarray(
                np.concatenate([x_tab16[ia], d_tab16[ia]], axis=1)),
            "en_b": np.ascontiguousarray(
                np.concatenate([x_tab16[ib], d_tab16[ib]], axis=1)),
            "po": np.ascontiguousarray(
                np.concatenate([xs_c[c], dinv_c[c]], axis=1)),
        })
    res1 = run_bass_kernel_spmd(nc1, maps1, list(range(NCORES)), trace=trace)

    # host routes layer-1 message values to edge slots (halo exchange)
    w_tab = np.zeros(SENT + 1, dtype=np.float32)
    w_own_c = []
    for c in range(NCORES):
        w = np.asarray(res1.results[c]["out"])
        w_own_c.append(np.ascontiguousarray(w.astype(np.float32)))
        w_tab[c * NPC:(c + 1) * NPC] = w.reshape(-1)
    w_tab16 = w_tab.astype(F16)

    # ---- layer 2 ----
    nc2 = _build_layer2(K, b2v)
    maps2 = [{
        "we_a": np.ascontiguousarray(w_tab16[idx_c[c][:, :cA]]),
        "we_b": np.ascontiguousarray(w_tab16[idx_c[c][:, cA:]]),
        "po": np.ascontiguousarray(
            np.concatenate([w_own_c[c], dinv_c[c]], axis=1)),
    } for c in range(NCORES)]
    res2 = run_bass_kernel_spmd(nc2, maps2, list(range(NCORES)), trace=trace)

    LAST_RESULTS = [res1, res2]

    out = np.empty((N, 1), dtype=np.float32)
    for c in range(NCORES):
        lo, hi = c * NPC, min((c + 1) * NPC, N)
        o_sorted = np.asarray(res2.results[c]["out"]).reshape(NPC)
        out[lo:hi, 0] = o_sorted[rank_c[c][:hi - lo]]
    return out


# revision 13
# speedup vs baseline: 1.1603x; 1.1603x over previous
"""GCN (2-layer, hidden=64, rank-1 weights) on 8 Trainium2 NeuronCores.

Math: both GCNConv layers have rank-1 weight matrices (1->64, 64->1), so each
layer collapses to a scalar SpMV with the symmetric-normalized adjacency
A_hat = D^-1/2 (A+I) D^-1/2:

    s   = A_hat @ x                    (scalar per node)
    z   = f(s)   where f(t) = sum_k W2[k] * relu(W1[k]*t + b1[k])
    out = A_hat @ z + b2

Sharding: nodes are range-sharded by destination across the 8 cores; all
in-edges of a node live on its owner core.  Within a core, nodes are sorted
by in-degree (descending) so that "round r" (the r-th in-edge of every node
that has one) is a dense prefix of node slots -- the edge-routed per-slot
value arrays are therefore nearly pad-free (ELL with degree-sorted rounds).

Execution is two SPMD launches (one per GCN layer).  The host routes
per-edge source data to the owning destination core between layers (np.take
-- pure gather, the "halo exchange" of the sharding strategy).  Routed
per-edge tables are fp16 so the DVE's 2-byte fast path applies and HBM
traffic is halved.  Normalization coefficients dinv = 1/sqrt(deg+1) are a
function of the graph structure only (host already derives degrees with
np.bincount to build the routing tables), so the host routes dinv[src]
per-edge directly; the device performs all feature arithmetic: the per-edge
message products dinv[src]*x[src], the segment summation (fold-tree reduce
over the ELL tile), the MLP nonlinearity (weight-folded to a 2-segment
piecewise-linear map when b1 == 0), the per-node normalization, the layer-2
message values w = dinv*z, and the bias.

Only the vector (DVE) and sync engines are used: relu is a fused
tensor_scalar (max,mult) on DVE, so no ACT table load, and intra-engine
ordering relies on the DVE's in-order execution (no per-op semaphore churn).
Each layer's routed table is DMA'd in four column chunks with per-chunk
semaphores so each chunk's fold overlaps the next chunk's transfer.
"""

import os
import numpy as np

from concourse import bass, mybir
from concourse.bass_utils import run_bass_kernel_spmd

dt = mybir.dt
F16 = np.float16

NCORES = 8
N = 100000
P = 128            # SBUF partitions
CPN = 98           # node columns per partition
NPC = P * CPN      # 12544 nodes per core
SENT = NCORES * NPC  # sentinel table slot (value 0)

LAST_RESULTS = None  # list of BassKernelResults from the most recent run


def _chunks(K):
    """Split K ELL round-blocks into up to 4 pipeline chunks.
    Returns a list of (first_block, n_blocks)."""
    n = min(4, K)
    base = K // n
    sizes = [K - base * (n - 1)] + [base] * (n - 1)
    out, b = [], 0
    for s in sizes:
        out.append((b, s))
        b += s
    return out


def _preprocess(x, edge_index):
    """Host routing/layout: shard by destination, degree-sort nodes, build
    per-slot source-index arrays (ELL with degree-sorted rounds)."""
    x = np.asarray(x, dtype=np.float32).reshape(-1)
    ei = np.asarray(edge_index)
    src_g = ei[0].astype(np.int64)
    dst_g = ei[1].astype(np.int64)

    cnt_g = np.bincount(dst_g, minlength=N).astype(np.int64)  # in-degree

    order_c, rank_c, deg_sorted_c = [], [], []
    pp = np.empty(N, dtype=np.int64)  # global node -> permuted table position
    for c in range(NCORES):
        lo, hi = c * NPC, min((c + 1) * NPC, N)
        nreal = hi - lo
        deg_local = np.zeros(NPC, dtype=np.int64)
        deg_local[:nreal] = cnt_g[lo:hi]
        order = np.argsort(-deg_local, kind="stable")
        rank = np.empty(NPC, dtype=np.int64)
        rank[order] = np.arange(NPC)
        order_c.append(order)
        rank_c.append(rank)
        deg_sorted_c.append(deg_local[order])
        pp[lo:hi] = c * NPC + rank[:nreal]

    K = int(max(int(d[0]) for d in deg_sorted_c))  # global max in-degree

    owner = dst_g // NPC
    idx_c, xs_c, dinv_c = [], [], []
    for c in range(NCORES):
        lo = c * NPC
        m = owner == c
        s_e = pp[src_g[m]]
        d_e = dst_g[m] - lo
        rj = rank_c[c][d_e]
        o = np.argsort(rj, kind="stable")
        rj_s = rj[o]
        s_s = s_e[o]
        occ = np.arange(len(rj_s)) - np.searchsorted(rj_s, rj_s)
        idx_mat = np.full((NPC, K), SENT, dtype=np.int64)
        idx_mat[rj_s, occ] = s_s
        # SBUF layout [p, r*98 + cc] for node j = p*98 + cc
        idx_c.append(np.ascontiguousarray(
            idx_mat.reshape(P, CPN, K).transpose(0, 2, 1).reshape(P, K * CPN)))

        nreal = min(NPC, N - lo)
        xv = np.zeros(NPC, dtype=np.float32)
        xv[:nreal] = x[lo:lo + nreal]
        xs_c.append(np.ascontiguousarray(
            xv[order_c[c]].astype(np.float32).reshape(P, CPN)))
        dinv_c.append(np.ascontiguousarray(
            (1.0 / np.sqrt(deg_sorted_c[c] + 1.0)).astype(np.float32)
            .reshape(P, CPN)))
    return idx_c, xs_c, dinv_c, rank_c, K


def _emit_tree(vector, Y, lo, nb):
    """In-place fold-tree over `nb` CPN-wide column blocks of Y starting at
    block `lo`; the sum lands in block `lo`.  All slices are contiguous."""
    w = nb
    while w > 1:
        h = (w + 1) // 2
        vector.tensor_tensor(
            out=Y[:, lo * CPN:(lo + w - h) * CPN],
            in0=Y[:, lo * CPN:(lo + w - h) * CPN],
            in1=Y[:, (lo + h) * CPN:(lo + w) * CPN],
            op=mybir.AluOpType.add)
        w = h


def _emit_combine(vector, Y, chunks):
    """Pairwise-merge the per-chunk partial sums (at each chunk's first
    block) into the first chunk's first block, all fp16."""
    heads = [b0 for (b0, _) in chunks]
    while len(heads) > 1:
        nxt = []
        for i in range(0, len(heads) - 1, 2):
            a, b = heads[i], heads[i + 1]
            vector.tensor_tensor(
                out=Y[:, a * CPN:(a + 1) * CPN],
                in0=Y[:, a * CPN:(a + 1) * CPN],
                in1=Y[:, b * CPN:(b + 1) * CPN],
                op=mybir.AluOpType.add)
            nxt.append(a)
        if len(heads) % 2:
            nxt.append(heads[-1])
        heads = nxt


def _build_layer1(K, A, B, terms):
    """Layer 1: routed per-edge tables [x[src] | dinv[src]] (fp16, in four
    column chunks), per-node [x_own | dinv | dinv^2] (f32).
    Output: w_own = dinv * f(s)  [the routed message value for layer 2]."""
    nc = bass.Bass(num_devices=NCORES)
    chunks = _chunks(K)

    en_in = [nc.declare_dram_parameter(f"en{i}", [P, 2 * nb * CPN], dt.float16,
                                       isOutput=False)
             for i, (_, nb) in enumerate(chunks)]
    po_in = nc.declare_dram_parameter("po", [P, 3 * CPN], dt.float32, isOutput=False)
    out_ext = nc.declare_dram_parameter("out", [P, CPN], dt.float32, isOutput=True)

    with (
        nc.sbuf_tensor("E0", [P, 2 * chunks[0][1] * CPN], dt.float16) as E0,
        nc.sbuf_tensor("E1", [P, 2 * chunks[min(1, len(chunks) - 1)][1] * CPN],
                       dt.float16) as E1,
        nc.sbuf_tensor("E2", [P, 2 * chunks[min(2, len(chunks) - 1)][1] * CPN],
                       dt.float16) as E2,
        nc.sbuf_tensor("E3", [P, 2 * chunks[min(3, len(chunks) - 1)][1] * CPN],
                       dt.float16) as E3,
        nc.sbuf_tensor("Y", [P, K * CPN], dt.float16) as Y,
        nc.sbuf_tensor("PO", [P, 3 * CPN], dt.float32) as PO,
        nc.sbuf_tensor("tb", [P, CPN], dt.float32) as tb,
        nc.sbuf_tensor("tr", [P, CPN], dt.float32) as tr,
        nc.sbuf_tensor("to", [P, CPN], dt.float32) as to,
        nc.semaphore("s0") as s0,
        nc.semaphore("s1") as s1,
        nc.semaphore("s2") as s2,
        nc.semaphore("s3") as s3,
        nc.semaphore("sp") as sp,
        nc.semaphore("sv") as sv,
        nc.Block() as block,
    ):
        E = [E0, E1, E2, E3][:len(chunks)]
        S = [s0, s1, s2, s3][:len(chunks)]

        @block.vector
        def _(vector):
            xo = PO[:, 0:CPN]
            do = PO[:, CPN:2 * CPN]
            dd = PO[:, 2 * CPN:3 * CPN]
            for i, (b0, nb) in enumerate(chunks):
                c = nb * CPN
                vector.wait_ge(S[i], 16)
                # per-edge messages y = dinv[src] * x[src], all fp16
                vector.tensor_tensor(
                    out=Y[:, b0 * CPN:b0 * CPN + c],
                    in0=E[i][:, 0:c], in1=E[i][:, c:2 * c],
                    op=mybir.AluOpType.mult)
                _emit_tree(vector, Y, b0, nb)
            _emit_combine(vector, Y, chunks)
            # t = fold + dinv * x_own  (s = dinv * t)
            vector.wait_ge(sp, 16)
            vector.tensor_tensor(out=tb[:, :], in0=do, in1=xo,
                                 op=mybir.AluOpType.mult)
            vector.tensor_tensor(out=tb[:, :], in0=tb[:, :], in1=Y[:, 0:CPN],
                                 op=mybir.AluOpType.add)
            if terms is None:
                # w = dinv*z = dinv^2 * ((A-B)*relu(t) + B*t)
                #   (relu(dinv*t) = dinv*relu(t) since dinv > 0)
                vector.tensor_scalar(tr[:, :], tb[:, :], 0.0, float(A - B),
                                     mybir.AluOpType.max,
                                     mybir.AluOpType.mult)
                vector.scalar_tensor_tensor(
                    out=tr[:, :], in0=tb[:, :], scalar=float(B), in1=tr[:, :],
                    op0=mybir.AluOpType.mult, op1=mybir.AluOpType.add)
                vector.tensor_tensor(
                    out=to[:, :], in0=dd, in1=tr[:, :],
                    op=mybir.AluOpType.mult).then_inc(sv, 1)
            else:
                # general path: s = dinv*t, z = sum_k W2k*relu(W1k*s+b1k)
                vector.tensor_tensor(out=tb[:, :], in0=do, in1=tb[:, :],
                                     op=mybir.AluOpType.mult)
                vector.memset(to[:, :], 0.0)
                for (w1k, b1k, w2k) in terms:
                    vector.tensor_scalar(
                        tr[:, :], tb[:, :], float(w1k), float(b1k),
                        mybir.AluOpType.mult, mybir.AluOpType.add)
                    vector.tensor_scalar_max(tr[:, :], tr[:, :], 0.0)
                    vector.scalar_tensor_tensor(
                        out=to[:, :], in0=tr[:, :], scalar=float(w2k),
                        in1=to[:, :],
                        op0=mybir.AluOpType.mult, op1=mybir.AluOpType.add)
                vector.tensor_tensor(
                    out=to[:, :], in0=do, in1=to[:, :],
                    op=mybir.AluOpType.mult).then_inc(sv, 1)

        @block.sync
        def _(sync):
            for i in range(len(chunks)):
                sync.dma_start(out=E[i][:, :], in_=en_in[i][:, :]).then_inc(S[i], 16)
            sync.dma_start(out=PO[:, :], in_=po_in[:, :]).then_inc(sp, 16)
            sync.wait_ge(sv, 1)
            sync.dma_start(out=out_ext[:, :], in_=to[:, :]).then_inc(sp, 16)

    return nc


def _build_layer2(K, b2val):
    """Layer 2: routed per-edge table w[src] (fp16, four column chunks; w is
    the device-computed dinv*z), per-node [w_own | dinv] (f32).
    out = dinv*(sum w_ell + w_own) + b2."""
    nc = bass.Bass(num_devices=NCORES)
    chunks = _chunks(K)

    we_in = [nc.declare_dram_parameter(f"we{i}", [P, nb * CPN], dt.float16,
                                       isOutput=False)
             for i, (_, nb) in enumerate(chunks)]
    po_in = nc.declare_dram_parameter("po", [P, 2 * CPN], dt.float32, isOutput=False)
    out_ext = nc.declare_dram_parameter("out", [P, CPN], dt.float32, isOutput=True)

    with (
        nc.sbuf_tensor("Y", [P, K * CPN], dt.float16) as Y,
        nc.sbuf_tensor("PO", [P, 2 * CPN], dt.float32) as PO,
        nc.sbuf_tensor("tb", [P, CPN], dt.float32) as tb,
        nc.sbuf_tensor("to", [P, CPN], dt.float32) as to,
        nc.semaphore("s0") as s0,
        nc.semaphore("s1") as s1,
        nc.semaphore("s2") as s2,
        nc.semaphore("s3") as s3,
        nc.semaphore("sp") as sp,
        nc.semaphore("sv") as sv,
        nc.Block() as block,
    ):
        S = [s0, s1, s2, s3][:len(chunks)]

        @block.vector
        def _(vector):
            wo = PO[:, 0:CPN]
            do = PO[:, CPN:2 * CPN]
            for i, (b0, nb) in enumerate(chunks):
                vector.wait_ge(S[i], 16)
                _emit_tree(vector, Y, b0, nb)
            _emit_combine(vector, Y, chunks)
            vector.wait_ge(sp, 16)
            vector.tensor_tensor(out=tb[:, :], in0=Y[:, 0:CPN], in1=wo,
                                 op=mybir.AluOpType.add)
            vector.tensor_tensor(out=tb[:, :], in0=do, in1=tb[:, :],
                                 op=mybir.AluOpType.mult)
            vector.tensor_scalar_add(to[:, :], tb[:, :],
                                     float(b2val)).then_inc(sv, 1)

        @block.sync
        def _(sync):
            for i, (b0, nb) in enumerate(chunks):
                sync.dma_start(
                    out=Y[:, b0 * CPN:(b0 + nb) * CPN],
                    in_=we_in[i][:, :]).then_inc(S[i], 16)
            sync.dma_start(out=PO[:, :], in_=po_in[:, :]).then_inc(sp, 16)
            sync.wait_ge(sv, 1)
            sync.dma_start(out=out_ext[:, :], in_=to[:, :]).then_inc(sp, 16)

    return nc


def kernel(x, edge_index, W1, b1, W2, b2):
    global LAST_RESULTS
    idx_c, xs_c, dinv_c, rank_c, K = _preprocess(x, edge_index)
    chunks = _chunks(K)

    w1 = np.asarray(W1, dtype=np.float64).reshape(-1)
    w2 = np.asarray(W2, dtype=np.float64).reshape(-1)
    b1v = np.asarray(b1, dtype=np.float64).reshape(-1)
    b2v = float(np.asarray(b2, dtype=np.float64).reshape(-1)[0])
    if np.all(b1v == 0.0):
        A = float(np.sum(w2 * w1 * (w1 > 0)))
        B = float(np.sum(w2 * w1 * (w1 < 0)))
        terms = None
    else:
        A = B = 0.0
        terms = [(float(w1[k]), float(b1v[k]), float(w2[k]))
                 for k in range(len(w1))]

    # routed tables in permuted (per-core degree-sorted) order + sentinel 0
    x_tab = np.zeros(SENT + 1, dtype=np.float32)
    d_tab = np.zeros(SENT + 1, dtype=np.float32)
    for c in range(NCORES):
        x_tab[c * NPC:(c + 1) * NPC] = xs_c[c].reshape(-1)
        d_tab[c * NPC:(c + 1) * NPC] = dinv_c[c].reshape(-1)
    x_tab16 = x_tab.astype(F16)
    d_tab16 = d_tab.astype(F16)

    trace = bool(os.environ.get("BASS_TRACE"))

    # ---- layer 1 ----
    nc1 = _build_layer1(K, A, B, terms)
    maps1 = []
    for c in range(NCORES):
        m = {}
        for i, (b0, nb) in enumerate(chunks):
            cols = idx_c[c][:, b0 * CPN:(b0 + nb) * CPN]
            m[f"en{i}"] = np.ascontiguousarray(
                np.concatenate([x_tab16[cols], d_tab16[cols]], axis=1))
        m["po"] = np.ascontiguousarray(
            np.concatenate([xs_c[c], dinv_c[c], dinv_c[c] * dinv_c[c]],
                           axis=1))
        maps1.append(m)
    res1 = run_bass_kernel_spmd(nc1, maps1, list(range(NCORES)), trace=trace)

    # host routes layer-1 message values to edge slots (halo exchange)
    w_tab = np.zeros(SENT + 1, dtype=np.float32)
    w_own_c = []
    for c in range(NCORES):
        w = np.asarray(res1.results[c]["out"])
        w_own_c.append(np.ascontiguousarray(w.astype(np.float32)))
        w_tab[c * NPC:(c + 1) * NPC] = w.reshape(-1)
    w_tab16 = w_tab.astype(F16)

    # ---- layer 2 ----
    nc2 = _build_layer2(K, b2v)
    maps2 = []
    for c in range(NCORES):
        m = {}
        for i, (b0, nb) in enumerate(chunks):
            cols = idx_c[c][:, b0 * CPN:(b0 + nb) * CPN]
            m[f"we{i}"] = np.ascontiguousarray(w_tab16[cols])
        m["po"] = np.ascontiguousarray(
            np.concatenate([w_own_c[c], dinv_c[c]], axis=1))
        maps2.append(m)
    res2 = run_bass_kernel_spmd(nc2, maps2, list(range(NCORES)), trace=trace)

    LAST_RESULTS = [res1, res2]

    out = np.empty((N, 1), dtype=np.float32)
    for c in range(NCORES):
        lo, hi = c * NPC, min((c + 1) * NPC, N)
        o_sorted = np.asarray(res2.results[c]["out"]).reshape(NPC)
        out[lo:hi, 0] = o_sorted[rank_c[c][:hi - lo]]
    return out
